# revision 1
# baseline (speedup 1.0000x reference)
import sys
sys.path.insert(0, "/opt/trn_rl_repo")
import numpy as np
from contextlib import ExitStack

from concourse import bacc, mybir, tile
from concourse import bass_utils
from concourse.masks import make_identity

# nn_MultiHeadAttention: B=4, T=2048, C=1024, H=16, HS=64
# Sharding: core = 2*b + hh; each core handles batch b, heads hh*8..hh*8+7.
# Per-core output is a partial [T, C] (its 8 heads through Wproj rows);
# host sums the pair (hh=0,1) per batch. Bias folded into hh==0 cores.
#
# v2: bf16 datapath with fp8(DoubleRow) on selected projection paths.
# Heads processed in pairs (even head on partitions 0-63, odd on 64-127):
# S = K^T Q uses row-tiled (tile_position) matmul pairs (concurrent on HW),
# PV and the softmax row-sum l use col-tiled pairs; l's stationary is a
# [128, 64] ones tile so 1/l lands aligned with the PV rows it normalizes.
# exp runs on Act from 2-bank PSUM tiles, skipping fully-masked causal
# blocks; S matmuls are narrowed to unmasked columns on diagonal tiles.

B, T, C = 4, 2048, 1024
H, HS = 16, 64
HL = 8            # local heads per core
NP = HL // 2      # head pairs per core
W = HL * HS       # 512
SCALE = 1.0 / 32.0  # C ** -0.5

# fp8-DoubleRow per path (weights host-prescaled by 16 to avoid the
# fp8e4m3 subnormal range; scale corrections folded into free scalars)
FP8_QK = True     # x8/wqk8 -> S path (softmax damps the error)
FP8_V = False     # x8/wv8  -> v path (error flows straight to output)
FP8_P = False     # out_T8/wp8 -> proj

F32 = mybir.dt.float32
F32R = mybir.dt.float32r
BF16 = mybir.dt.bfloat16
FP8 = mybir.dt.float8e4
DRMODE = mybir.MatmulPerfMode.DoubleRow
AF = mybir.ActivationFunctionType
ALU = mybir.AluOpType

_NC = {}


def _build(repeat=1):
    need_x8 = FP8_QK or FP8_V
    need_xb = not (FP8_QK and FP8_V)
    sot = 16.0 if FP8_P else 1.0            # out_T holds sot * attn
    nrm_scal = sot / (16.0 if FP8_V else 1.0)
    prj_scal = 1.0 / (sot * (16.0 if FP8_P else 1.0))
    exp_scal = SCALE / (256.0 if FP8_QK else 1.0)

    nc = bacc.Bacc("TRN2", target_bir_lowering=False, debug=False, num_devices=1)
    xb_ap = nc.dram_tensor("XB", (T, C), BF16, kind="ExternalInput").ap()
    # WQK [pair, kq, C, 128]; kq=0 -> [Wk_even|Wk_odd], kq=1 -> [Wq_even|Wq_odd]
    wqk_ap = nc.dram_tensor("WQK", (NP * 2 * C, 128),
                            FP8 if FP8_QK else BF16,
                            kind="ExternalInput").ap()
    wv_ap = nc.dram_tensor("WV", (C, W), FP8 if FP8_V else BF16,
                           kind="ExternalInput").ap()
    wp_ap = nc.dram_tensor("WP", (W, C), FP8 if FP8_P else BF16,
                           kind="ExternalInput").ap()
    bias_ap = nc.dram_tensor("BIAS", (1, C), F32, kind="ExternalInput").ap()
    # AMASK [128, j, par(2), 512] multiplicative causal mask, par-duplicated
    amask_ap = nc.dram_tensor("AMASK", (128, 4, 2, 512), BF16,
                              kind="ExternalInput").ap()
    out_ap = nc.dram_tensor("OUT", (T, C), F32, kind="ExternalOutput").ap()

    with tile.TileContext(nc) as tc, ExitStack() as ctx:
        pers = ctx.enter_context(tc.tile_pool(name="pers", bufs=1))
        ident_sb = pers.tile([128, 128], BF16)
        amask_sb = pers.tile([128, 4, 2, 512], BF16)
        bias_sb = pers.tile([1, C], F32R)
        ones_row = pers.tile([1, 128], F32R)
        ones64 = pers.tile([128, 64], BF16)        # l stationary
        # four separate full tiles (one per t-quarter): XBAR dma transpose
        # corrupts data on HW when its destination is a slice of a larger
        # tile, so give each transpose its own whole tile.
        xTs = [pers.tile([128, 8, 512], BF16, name=f"xT{g}")
               for g in range(4)]
        if need_x8:
            xT8s = [pers.tile([128, 8, 512], FP8, name=f"xT8{g}")
                    for g in range(4)]
        v_sb = pers.tile([128, 16, W], BF16)       # [s-in-tile, tt, h*64+d]
        kqT = pers.tile([128, NP, 2, T], BF16)     # [par*64+d, pair, k/q, t]
        out_T = pers.tile([128, 4, T], FP8 if FP8_P else BF16)

        make_identity(nc, ident_sb)
        nc.gpsimd.dma_start(amask_sb, amask_ap)
        nc.scalar.dma_start(bias_sb, bias_ap.bitcast(F32R))
        nc.scalar.activation(ones_row, ident_sb[0:1, :], AF.Copy,
                             bias=1.0, scale=0.0)
        nc.scalar.activation(ones64, ident_sb[:, 0:64], AF.Copy,
                             bias=1.0, scale=0.0)

        for _rep in range(repeat):
            wqk_r = wqk_ap.rearrange("(pr kq ct p) m -> pr p kq ct m",
                                     pr=NP, kq=2, p=128)
            with tc.tile_pool(name="wv", bufs=1) as wv_pool, \
                 tc.tile_pool(name="wqk", bufs=2) as wqk_pool, \
                 tc.tile_pool(name="pp", bufs=5) as p_pool, \
                 tc.tile_pool(name="nrm", bufs=2) as n_pool, \
                 tc.tile_pool(name="ostg", bufs=2) as o_pool, \
                 tc.tile_pool(name="psA", bufs=2, space="PSUM") as psA, \
                 tc.tile_pool(name="pss", bufs=2, space="PSUM") as ps_s, \
                 tc.tile_pool(name="pspv", bufs=1, space="PSUM") as ps_pv, \
                 tc.tile_pool(name="psl", bufs=1, space="PSUM") as ps_l:

                # ---- Phase 1: xT = x^T via XBAR DMA transpose ----
                # writes xT[p, ct, t] = x[t, ct*128 + p]; fp8 copy of it
                # via gpsimd casting DMA.
                wv_sb = wv_pool.tile([128, 8, W], FP8 if FP8_V else BF16)
                nc.gpsimd.dma_start(
                    wv_sb, wv_ap.rearrange("(ct p) n -> p ct n", p=128))
                wp_sb = wv_pool.tile([128, 4, C], FP8 if FP8_P else BF16)
                nc.scalar.dma_start(
                    wp_sb, wp_ap.rearrange("(ct p) n -> p ct n", p=128))
                # XBAR transposes must serialize on ONE queue: concurrent
                # dma_start_transpose on different queues corrupts (shared
                # XBAR block, verified on HW)
                for g in range(4):
                    nc.sync.dma_start_transpose(
                        xTs[g], xb_ap[g * 512:(g + 1) * 512, :])
                if need_x8:
                    for g in range(4):
                        nc.gpsimd.dma_start(xT8s[g], xTs[g])

                # bias broadcast [128, C] via PE (2 matmuls) + DVE staging
                bias_bc = wv_pool.tile([128, C], F32)
                for ch in range(2):
                    pbb = psA.tile([128, 512], F32, tag="scr")
                    nc.tensor.matmul(pbb, ones_row,
                                     bias_sb[:, ch * 512:(ch + 1) * 512],
                                     start=True, stop=True)
                    nc.vector.tensor_copy(bias_bc[:, ch * 512:(ch + 1) * 512],
                                          pbb)

                # ---- Phase 2: v = x @ Wv ----
                for tt in range(16):
                    pv_ = psA.tile([128, W], F32, tag="scr")
                    if FP8_V:
                        for g in range(4):
                            nc.tensor.matmul(
                                pv_,
                                xT8s[tt // 4][:, 2 * g:2 * g + 2,
                                              (tt % 4) * 128:
                                              (tt % 4 + 1) * 128],
                                wv_sb[:, 2 * g:2 * g + 2, :],
                                start=(g == 0), stop=(g == 3),
                                perf_mode=DRMODE)
                    else:
                        for ct in range(8):
                            nc.tensor.matmul(
                                pv_,
                                xTs[tt // 4][:, ct, (tt % 4) * 128:
                                             (tt % 4 + 1) * 128],
                                wv_sb[:, ct, :],
                                start=(ct == 0), stop=(ct == 7))
                    nc.vector.tensor_copy(v_sb[:, tt], pv_)

                # ---- Phase 3: per-pair QK^T proj, attention ----
                for pair in range(NP):
                    wqk_sb = wqk_pool.tile([128, 2, 8, 128],
                                           FP8 if FP8_QK else BF16)
                    nc.gpsimd.dma_start(wqk_sb, wqk_r[pair])
                    for kq in range(2):
                        for tc4 in range(4):
                            pqk = psA.tile([128, 512], F32, tag="scr")
                            if FP8_QK:
                                for g in range(4):
                                    nc.tensor.matmul(
                                        pqk,
                                        wqk_sb[:, kq, 2 * g:2 * g + 2, :],
                                        xT8s[tc4][:, 2 * g:2 * g + 2, :],
                                        start=(g == 0), stop=(g == 3),
                                        perf_mode=DRMODE)
                            else:
                                for ct in range(8):
                                    nc.tensor.matmul(
                                        pqk, wqk_sb[:, kq, ct, :],
                                        xTs[tc4][:, ct, :],
                                        start=(ct == 0), stop=(ct == 7))
                            nc.vector.tensor_copy(
                                kqT[:, pair, kq, tc4 * 512:(tc4 + 1) * 512],
                                pqk)

                    for tc4 in range(4):
                        n_s = 4 * (tc4 + 1)
                        pv = ps_pv.tile([128, 512], F32)
                        pl = ps_l.tile([128, 512], F32)
                        for st in range(n_s):
                            ps = ps_s.tile([128, 2, 512], F32)
                            j = st - 4 * tc4
                            c0 = max(j, 0) * 128
                            for par in range(2):
                                nc.tensor.matmul(
                                    ps[:, par, c0:],
                                    kqT[par * 64:par * 64 + 64, pair, 0,
                                        st * 128:(st + 1) * 128],
                                    kqT[par * 64:par * 64 + 64, pair, 1,
                                        tc4 * 512 + c0:(tc4 + 1) * 512],
                                    start=True, stop=True)
                            p_t = p_pool.tile([128, 2, 512], BF16)
                            if c0 > 0:
                                nc.gpsimd.memset(p_t[:, :, 0:c0], 0.0)
                            nc.scalar.activation(p_t[:, :, c0:],
                                                 ps[:, :, c0:], AF.Exp,
                                                 bias=0.0, scale=exp_scal)
                            if j >= 0:
                                nc.vector.tensor_tensor(
                                    p_t[:, :, c0:c0 + 128],
                                    p_t[:, :, c0:c0 + 128],
                                    amask_sb[:, j, :, c0:c0 + 128],
                                    ALU.mult)
                            st_f, st_l = (st == 0), (st == n_s - 1)
                            for par in range(2):
                                h = 2 * pair + par
                                nc.tensor.matmul(
                                    pv[par * 64:par * 64 + 64, c0:],
                                    v_sb[:, st, h * 64:h * 64 + 64],
                                    p_t[:, par, c0:],
                                    start=st_f, stop=st_l,
                                    skip_group_check=True)
                            for par in range(2):
                                nc.tensor.matmul(
                                    pl[par * 64:par * 64 + 64, c0:],
                                    ones64, p_t[:, par, c0:],
                                    start=st_f, stop=st_l,
                                    skip_group_check=True)
                        rcl = n_pool.tile([128, 512], F32)
                        nc.vector.reciprocal(rcl, pl)
                        nc.vector.scalar_tensor_tensor(
                            out_T[:, pair, tc4 * 512:(tc4 + 1) * 512],
                            pv, nrm_scal, rcl, ALU.mult, ALU.mult)

                        # ---- Phase 4 (interleaved): proj for ready tokens
                        if pair == NP - 1:
                            out_r = out_ap.rearrange("(tt p) n -> p tt n",
                                                     p=128)
                            for tt in range(tc4 * 4, tc4 * 4 + 4):
                                ostg = o_pool.tile([128, C], F32)
                                for ch in range(2):
                                    po = psA.tile([128, 512], F32, tag="scr")
                                    if FP8_P:
                                        for g in range(2):
                                            nc.tensor.matmul(
                                                po,
                                                out_T[:, 2 * g:2 * g + 2,
                                                      tt * 128:(tt + 1) * 128],
                                                wp_sb[:, 2 * g:2 * g + 2,
                                                      ch * 512:(ch + 1) * 512],
                                                start=(g == 0), stop=(g == 1),
                                                perf_mode=DRMODE)
                                    else:
                                        for ct in range(4):
                                            nc.tensor.matmul(
                                                po,
                                                out_T[:, ct,
                                                      tt * 128:(tt + 1) * 128],
                                                wp_sb[:, ct,
                                                      ch * 512:(ch + 1) * 512],
                                                start=(ct == 0),
                                                stop=(ct == 3))
                                    nc.vector.scalar_tensor_tensor(
                                        ostg[:, ch * 512:(ch + 1) * 512],
                                        po, prj_scal,
                                        bias_bc[:, ch * 512:(ch + 1) * 512],
                                        ALU.mult, ALU.add)
                                nc.sync.dma_start(out_r[:, tt, :], ostg)

    nc.finalize()
    return nc


def _in_maps(inputs):
    import ml_dtypes
    bf16 = ml_dtypes.bfloat16
    fp8 = ml_dtypes.float8_e4m3
    x = np.asarray(inputs["x"], dtype=np.float32)
    Wq = np.asarray(inputs["Wq"], dtype=np.float32)
    Wk = np.asarray(inputs["Wk"], dtype=np.float32)
    Wv = np.asarray(inputs["Wv"], dtype=np.float32)
    Wp = np.asarray(inputs["Wproj"], dtype=np.float32)
    bp = np.asarray(inputs["bproj"], dtype=np.float32)

    s = np.arange(128)[:, None, None]
    j = np.arange(4)[None, :, None]
    tf = np.arange(512)[None, None, :]
    amask = np.where(128 * j + s > tf, np.float32(0.0), np.float32(1.0))
    amask = np.repeat(amask[:, :, None, :], 2, axis=2)  # par-duplicated
    amask = np.ascontiguousarray(amask.astype(bf16))

    maps = []
    for core in range(8):
        b, hh = core // 2, core % 2
        hs0 = hh * HL
        # WQK [pair, kq, C, 128]: kq=0 Wk pair, kq=1 Wq pair
        wqk = np.empty((NP, 2, C, 128), dtype=np.float32)
        for pr in range(NP):
            he, ho = hs0 + 2 * pr, hs0 + 2 * pr + 1
            wqk[pr, 0, :, 0:64] = Wk[he]
            wqk[pr, 0, :, 64:128] = Wk[ho]
            wqk[pr, 1, :, 0:64] = Wq[he]
            wqk[pr, 1, :, 64:128] = Wq[ho]
        wqk = wqk.reshape(NP * 2 * C, 128)
        wqk = (wqk * 16.0).astype(fp8) if FP8_QK else wqk.astype(bf16)
        wv = Wv[hs0:hs0 + HL].transpose(1, 0, 2).reshape(C, W)
        wv = (wv * 16.0).astype(fp8) if FP8_V else wv.astype(bf16)
        wp = Wp[hh * W:(hh + 1) * W]
        wp = (wp * 16.0).astype(fp8) if FP8_P else wp.astype(bf16)
        bias = (bp if hh == 0 else np.zeros_like(bp)).reshape(1, C)
        maps.append({
            "XB": np.ascontiguousarray(x[b].astype(bf16)),
            "WQK": np.ascontiguousarray(wqk),
            "WV": np.ascontiguousarray(wv),
            "WP": np.ascontiguousarray(wp),
            "BIAS": np.ascontiguousarray(bias),
            "AMASK": amask,
        })
    return maps


def get_nc(repeat=1):
    key = repeat
    if key not in _NC:
        _NC[key] = _build(repeat)
    return _NC[key]


def run(inputs, trace=False):
    res = bass_utils.run_bass_kernel_spmd(
        get_nc(), _in_maps(inputs), core_ids=list(range(8)), trace=trace)
    outs = [res.results[c]["OUT"] for c in range(8)]
    out = np.stack([outs[2 * b] + outs[2 * b + 1] for b in range(B)])
    return out.astype(np.float32), res.exec_time_ns


def kernel(**inputs):
    return run(inputs, trace=False)[0]

